# revision 1
# baseline (speedup 1.0000x reference)
"""NNConv GNN message passing on 8 Trainium2 NeuronCores.

Strategy (edge-parallel, dst-sharded):
  - Nodes split into 8 aligned ranges of 12544 (N padded to 100352); each core
    owns the edges whose dst falls in its range (sorted by dst on host).
  - Edge MLP: z2 = relu(relu(attr@W1+b1)@W2+b2) computed once per core
    (transposed layout [64, E]) and cached to HBM; per update only the last
    layer (z2 @ W3 -> per-edge 16x16 matrix) is recomputed on PE.
  - Messages: Wedge[e,(i,o)] * h[src_e, i] via DVE broadcast-multiply; the
    i-reduction is folded into the scatter matmul (aggregate [slot,(i,o)] in
    PSUM per 128-node window, reduce over i once per window).
  - Scatter-add: one-hot S built on GPSIMD (iota vs dst_slot is_equal),
    PSUM-accumulated S^T @ P matmuls over each window's K edge tiles.
  - h update: h_new = aggr + h@root (per-window PE transpose + matmul).
  - Cross-core: AllGather of each core's h slice after updates 1 and 2.
  - Readout computed on own nodes; partial sums summed on host.
"""

import sys

sys.path.insert(0, "/opt/trn_rl_repo")

import numpy as np

from concourse import bacc, bass, mybir, tile
from concourse.bass_utils import run_bass_kernel_spmd
from concourse.masks import make_identity

P = 128
F32 = mybir.dt.float32
I32 = mybir.dt.int32

FULL_CFG = dict(
    n_nodes=100000,
    n_edges=800000,
    n_cores=8,
    f_node=8,
    f_edge=4,
    h=16,
    m1=64,
    m2=64,
    rl=128,
)


def _cfg_derived(cfg):
    nc_cores = cfg["n_cores"]
    n_pad = ((cfg["n_nodes"] + nc_cores * P - 1) // (nc_cores * P)) * (nc_cores * P)
    nodes_per_core = n_pad // nc_cores
    n_windows = nodes_per_core // P
    return n_pad, nodes_per_core, n_windows


def preprocess(cfg, x, edge_index, edge_attr):
    """Shard edges by dst range, sort, window-pad to uniform K tiles/window."""
    n_pad, npc, n_win = _cfg_derived(cfg)
    ncc = cfg["n_cores"]
    src = np.asarray(edge_index[:, 0], np.int64)
    dst = np.asarray(edge_index[:, 1], np.int64)
    attr = np.asarray(edge_attr, np.float32)

    order = np.argsort(dst, kind="stable")
    src_s, dst_s, attr_s = src[order], dst[order], attr[order]
    core_of = dst_s // npc
    win_of = (dst_s % npc) // P

    # tiles needed per (core, window)
    K = 1
    counts = np.zeros((ncc, n_win), np.int64)
    for c in range(ncc):
        m = core_of == c
        w, cnt = np.unique(win_of[m], return_counts=True)
        counts[c, w] = cnt
    K = max(1, int(np.ceil(counts.max() / P)))
    while (n_win * K * P) % 512 != 0:
        K += 1
    e_pad = n_win * K * P

    src_pad = np.zeros((ncc, e_pad), np.int32)
    slot_pad = np.full((ncc, e_pad), -1.0, np.float32)
    attr_pad = np.zeros((ncc, e_pad, cfg["f_edge"]), np.float32)
    for c in range(ncc):
        m = core_of == c
        sc, dc, ac, wc = src_s[m], dst_s[m], attr_s[m], win_of[m]
        # edges are sorted by dst so windows are contiguous runs
        starts = np.searchsorted(wc, np.arange(n_win))
        ends = np.searchsorted(wc, np.arange(n_win), side="right")
        for w in range(n_win):
            s0, s1 = starts[w], ends[w]
            o0 = w * K * P
            n = s1 - s0
            src_pad[c, o0 : o0 + n] = sc[s0:s1]
            slot_pad[c, o0 : o0 + n] = (dc[s0:s1] % npc) % P
            attr_pad[c, o0 : o0 + n] = ac[s0:s1]

    n_tiles = e_pad // P
    # tiled layouts: [P, n_tiles] with tile g partition p = edge g*P+p
    src_t = src_pad.reshape(ncc, n_tiles, P).transpose(0, 2, 1).copy()
    slot_t = slot_pad.reshape(ncc, n_tiles, P).transpose(0, 2, 1).copy()
    # attr transposed blocks [n_blocks, f_edge, 512]
    FB = 512
    nb = e_pad // FB
    attr_t = attr_pad.reshape(ncc, nb, FB, cfg["f_edge"]).transpose(0, 1, 3, 2).copy()
    return dict(K=K, e_pad=e_pad, n_tiles=n_tiles, src_t=src_t, slot_t=slot_t,
                attr_t=attr_t, n_blocks=nb)


def build_program(cfg, K, bi2, bj2):
    """Build the SPMD bass program. Returns (nc, input_names)."""
    n_pad, npc, n_win = _cfg_derived(cfg)
    ncc = cfg["n_cores"]
    H, FE, M1, M2, RL = cfg["h"], cfg["f_edge"], cfg["m1"], cfg["m2"], cfg["rl"]
    HH = H * H
    n_tiles = n_win * K
    FB = 512
    n_blocks = (n_tiles * P) // FB
    TPB = FB // P  # tiles per z2 block
    GB = 1
    for cand in (7, 5, 4, 3, 2):
        if n_win % cand == 0:
            GB = cand
            break

    nc = bacc.Bacc("TRN2", target_bir_lowering=False, debug=False,
                   num_devices=ncc)

    def inp(name, shape, dt=F32):
        return nc.dram_tensor(name, list(shape), dt, kind="ExternalInput").ap()

    attr_d = inp("attr_t", (n_blocks, FE, FB))
    src_d = inp("src_t", (P, n_tiles), I32)
    slot_d = inp("slot_t", (P, n_tiles))
    h0_full_d = inp("h0_full", (n_pad, H))
    h0_own_d = inp("h0_own", (P, n_win, H))
    w1_d = inp("W1", (FE, M1))
    b1_d = inp("b1", (M1, 1))
    w2_d = inp("W2", (M1, M2))
    b2_d = inp("b2", (M2, 1))
    w3_d = inp("W3", (M2, HH))
    root_d = inp("root", (H, H))
    wi1_d = inp("Wi1", (2 * H, RL))
    bi1_d = inp("bi1", (RL, 1))
    wi2_d = inp("Wi2", (RL, 1))
    wj1_d = inp("Wj1", (H, RL))
    bj1_d = inp("bj1", (RL, 1))
    wj2_d = inp("Wj2", (RL, 1))
    partial_d = nc.dram_tensor("partial", [1, 1], F32, kind="ExternalOutput").ap()

    with tile.TileContext(nc) as tc:
        with (
            tc.tile_pool(name="const", bufs=1) as cp,
            tc.tile_pool(name="dram", bufs=1, space="DRAM") as dram,
            tc.tile_pool(name="sb", bufs=1) as sb,
            tc.tile_pool(name="pha", bufs=3) as pha,
            tc.tile_pool(name="work", bufs=3) as work,
            tc.tile_pool(name="ro", bufs=2) as ro,
            tc.tile_pool(name="gat", bufs=3) as gat,
            tc.tile_pool(name="pp", bufs=2, space="PSUM") as pp,
            tc.tile_pool(name="pss", bufs=2, space="PSUM") as ps_small,
        ):
            # ---- constants ----
            ident = cp.tile([P, P], F32, tag="ident")
            make_identity(nc, ident[:])
            iota_i = cp.tile([P, P], I32, tag="iota_i")
            nc.gpsimd.iota(iota_i[:], pattern=[[1, P]], base=0, channel_multiplier=0)
            iota_f = cp.tile([P, P], F32, tag="iota_f")
            nc.vector.tensor_copy(out=iota_f[:], in_=iota_i[:])

            def load(tag, ap_d, shape, dt=F32):
                t = cp.tile(list(shape), dt, tag=tag)
                nc.sync.dma_start(out=t[:], in_=ap_d)
                return t

            w1 = load("w1", w1_d[:], (FE, M1))
            b1 = load("b1", b1_d[:], (M1, 1))
            w2 = load("w2", w2_d[:], (M1, M2))
            b2 = load("b2", b2_d[:], (M2, 1))
            w3 = load("w3", w3_d[:], (M2, HH))
            root = load("root", root_d[:], (H, H))
            wi1 = cp.tile([64, RL], F32, tag="wi1")
            nc.gpsimd.memset(wi1[:], 0.0)
            nc.sync.dma_start(out=wi1[:H, :], in_=wi1_d[:H, :])
            nc.sync.dma_start(out=wi1[32 : 32 + H, :], in_=wi1_d[H:, :])
            bi1 = load("bi1", bi1_d[:], (RL, 1))
            wi2 = load("wi2", wi2_d[:], (RL, 1))
            wj1 = load("wj1", wj1_d[:], (H, RL))
            bj1 = load("bj1", bj1_d[:], (RL, 1))
            wj2 = load("wj2", wj2_d[:], (RL, 1))
            src_sb = load("src", src_d[:], (P, n_tiles), I32)
            slot_sb = load("slot", slot_d[:], (P, n_tiles))
            h0own = load("h0own", h0_own_d[:], (P, n_win, H))

            h_a = sb.tile([P, n_win, H], F32, tag="h_a")
            h_b = sb.tile([P, n_win, H], F32, tag="h_b")

            z2_dram = dram.tile([M2, n_tiles * P], F32, tag="z2d")
            hown_dram = dram.tile([npc, H], F32, tag="hod")
            hfull1 = dram.tile([n_pad, H], F32, tag="hf1")
            hfull2 = dram.tile([n_pad, H], F32, tag="hf2")

            # ---- phase A: edge MLP layers 1-2, cache z2^T to HBM ----
            for b in range(n_blocks):
                at = pha.tile([FE, FB], F32, tag="attr")
                nc.sync.dma_start(out=at[:], in_=attr_d[b])
                z1p = pp.tile([M1, FB], F32, tag="bigA", space="PSUM")
                nc.tensor.matmul(out=z1p[:], lhsT=w1[:], rhs=at[:],
                                 start=True, stop=True)
                z1s = pha.tile([M1, FB], F32, tag="z1s")
                nc.scalar.activation(out=z1s[:], in_=z1p[:],
                                     func=mybir.ActivationFunctionType.Relu,
                                     bias=b1[:])
                z2p = pp.tile([M2, FB], F32, tag="bigB", space="PSUM")
                nc.tensor.matmul(out=z2p[:], lhsT=w2[:], rhs=z1s[:],
                                 start=True, stop=True)
                z2s = pha.tile([M2, FB], F32, tag="z2s")
                nc.vector.tensor_scalar(out=z2s[:], in0=z2p[:], scalar1=b2[:],
                                        scalar2=0.0, op0=mybir.AluOpType.add,
                                        op1=mybir.AluOpType.max)
                nc.sync.dma_start(
                    out=z2_dram[:, b * FB : (b + 1) * FB], in_=z2s[:]
                )

            # ---- 3 message-passing updates ----
            for u in range(3):
                h_cur = h0own if u == 0 else (h_a if u == 1 else h_b)
                h_new = h_a if u == 0 else (h_b if u == 1 else h_a)
                h_tab = h0_full_d if u == 0 else (hfull1 if u == 1 else hfull2)[:]

                for w in range(n_win):
                    aggp = pp.tile([P, HH], F32, tag="bigB", space="PSUM")
                    for t in range(K):
                        g = w * K + t
                        if g % TPB == 0:
                            z2blk = work.tile([M2, FB], F32, tag="z2blk")
                            nc.sync.dma_start(
                                out=z2blk[:],
                                in_=z2_dram[:, g * P : g * P + FB],
                            )
                        hsrc = gat.tile([P, H], F32, tag="hsrc")
                        nc.gpsimd.indirect_dma_start(
                            out=hsrc[:],
                            out_offset=None,
                            in_=h_tab,
                            in_offset=bass.IndirectOffsetOnAxis(
                                ap=src_sb[:, g : g + 1], axis=0
                            ),
                        )
                        wp = pp.tile([P, HH], F32, tag="bigA", space="PSUM")
                        nc.tensor.matmul(
                            out=wp[:],
                            lhsT=z2blk[:, (g % TPB) * P : (g % TPB + 1) * P],
                            rhs=w3[:], start=True, stop=True,
                        )
                        s_t = work.tile([P, P], F32, tag="s_t")
                        nc.vector.tensor_scalar(
                            out=s_t[:], in0=iota_f[:],
                            scalar1=slot_sb[:, g : g + 1], scalar2=None,
                            op0=mybir.AluOpType.is_equal,
                        )
                        p_t = work.tile([P, H, H], F32, tag="p_t")
                        nc.vector.tensor_tensor(
                            out=p_t[:],
                            in0=wp[:].rearrange("p (i o) -> p i o", i=H),
                            in1=hsrc[:].to_broadcast([P, H, H]),
                            op=mybir.AluOpType.mult,
                        )
                        nc.tensor.matmul(
                            out=aggp[:], lhsT=s_t[:],
                            rhs=p_t[:].rearrange("p i o -> p (i o)"),
                            start=(t == 0), stop=(t == K - 1),
                        )
                    # window epilogue: reduce over i, h_new = aggr + h@root
                    agg = work.tile([P, H], F32, tag="agg")
                    nc.vector.tensor_reduce(
                        out=agg[:],
                        in_=aggp[:].rearrange("p (i o) -> p o i", i=H),
                        axis=mybir.AxisListType.X, op=mybir.AluOpType.add,
                    )
                    tp = ps_small.tile([H, P], F32, tag="small", space="PSUM")
                    nc.tensor.transpose(out=tp[:], in_=h_cur[:, w, :],
                                        identity=ident[:])
                    hT = work.tile([H, P], F32, tag="hT")
                    nc.scalar.activation(out=hT[:], in_=tp[:],
                                         func=mybir.ActivationFunctionType.Copy)
                    hup = ps_small.tile([P, H], F32, tag="small2", space="PSUM")
                    nc.tensor.matmul(out=hup[:], lhsT=hT[:], rhs=root[:],
                                     start=True, stop=True)
                    nc.vector.tensor_tensor(out=h_new[:, w, :], in0=hup[:],
                                            in1=agg[:], op=mybir.AluOpType.add)

                if u < 2:
                    nc.sync.dma_start(
                        out=hown_dram[:].rearrange("(w p) o -> p w o", p=P),
                        in_=h_new[:],
                    )
                    nc.gpsimd.collective_compute(
                        "AllGather", mybir.AluOpType.bypass,
                        replica_groups=[list(range(ncc))],
                        ins=[hown_dram.opt()],
                        outs=[(hfull1 if u == 0 else hfull2).opt()],
                    )

            # ---- readout on own nodes ----
            h_fin = h_a  # after u=2 writes into h_a
            acc = sb.tile([1, P], F32, tag="acc")
            nc.gpsimd.memset(acc[:], 0.0)
            for w in range(n_win):
                tp1 = ps_small.tile([H, P], F32, tag="small", space="PSUM")
                nc.tensor.transpose(out=tp1[:], in_=h_fin[:, w, :],
                                    identity=ident[:])
                hcat = ro.tile([64, P], F32, tag="hcat")
                nc.gpsimd.memset(hcat[:], 0.0)
                nc.scalar.activation(out=hcat[:H, :], in_=tp1[:],
                                     func=mybir.ActivationFunctionType.Copy)
                tp2 = ps_small.tile([H, P], F32, tag="small", space="PSUM")
                nc.tensor.transpose(out=tp2[:], in_=h0own[:, w, :],
                                    identity=ident[:])
                nc.scalar.activation(out=hcat[32 : 32 + H, :], in_=tp2[:],
                                     func=mybir.ActivationFunctionType.Copy)
                g1p = pp.tile([RL, P], F32, tag="bigA", space="PSUM")
                nc.tensor.matmul(out=g1p[:], lhsT=wi1[:], rhs=hcat[:],
                                 start=True, stop=True)
                g1s = ro.tile([RL, P], F32, tag="g1s")
                nc.scalar.activation(out=g1s[:], in_=g1p[:],
                                     func=mybir.ActivationFunctionType.Relu,
                                     bias=bi1[:])
                g2p = ps_small.tile([1, P], F32, tag="small2", space="PSUM")
                nc.tensor.matmul(out=g2p[:], lhsT=wi2[:], rhs=g1s[:],
                                 start=True, stop=True)
                gate = ro.tile([1, P], F32, tag="gate")
                nc.scalar.activation(out=gate[:], in_=g2p[:],
                                     func=mybir.ActivationFunctionType.Sigmoid,
                                     bias=float(bi2))
                v1p = pp.tile([RL, P], F32, tag="bigB", space="PSUM")
                nc.tensor.matmul(out=v1p[:], lhsT=wj1[:], rhs=hcat[:H, :],
                                 start=True, stop=True)
                v1s = ro.tile([RL, P], F32, tag="v1s")
                nc.scalar.activation(out=v1s[:], in_=v1p[:],
                                     func=mybir.ActivationFunctionType.Relu,
                                     bias=bj1[:])
                v2p = ps_small.tile([1, P], F32, tag="small2", space="PSUM")
                nc.tensor.matmul(out=v2p[:], lhsT=wj2[:], rhs=v1s[:],
                                 start=True, stop=True)
                val = ro.tile([1, P], F32, tag="val")
                nc.scalar.activation(out=val[:], in_=v2p[:],
                                     func=mybir.ActivationFunctionType.Copy,
                                     bias=float(bj2))
                prod = ro.tile([1, P], F32, tag="prod")
                nc.vector.tensor_tensor(out=prod[:], in0=gate[:], in1=val[:],
                                        op=mybir.AluOpType.mult)
                nc.vector.tensor_tensor(out=acc[:], in0=acc[:], in1=prod[:],
                                        op=mybir.AluOpType.add)
            part = sb.tile([1, 1], F32, tag="part")
            nc.vector.tensor_reduce(out=part[:], in_=acc[:],
                                    axis=mybir.AxisListType.X,
                                    op=mybir.AluOpType.add)
            nc.sync.dma_start(out=partial_d[:], in_=part[:])

    nc.compile()
    return nc


def make_in_maps(cfg, pre, inputs):
    """Per-core input dicts for run_bass_kernel_spmd."""
    n_pad, npc, n_win = _cfg_derived(cfg)
    ncc = cfg["n_cores"]
    H, FN = cfg["h"], cfg["f_node"]
    x = np.asarray(inputs["x"], np.float32)
    h0 = np.zeros((n_pad, H), np.float32)
    h0[: x.shape[0], :FN] = x
    h0_own = h0.reshape(ncc, n_win, P, H).transpose(0, 2, 1, 3).copy()

    common = dict(
        h0_full=h0,
        W1=np.asarray(inputs["W1"], np.float32),
        b1=np.asarray(inputs["b1"], np.float32).reshape(-1, 1),
        W2=np.asarray(inputs["W2"], np.float32),
        b2=np.asarray(inputs["b2"], np.float32).reshape(-1, 1),
        W3=np.asarray(inputs["W3"], np.float32),
        root=np.asarray(inputs["root"], np.float32),
        Wi1=np.asarray(inputs["Wi1"], np.float32),
        bi1=np.asarray(inputs["bi1"], np.float32).reshape(-1, 1),
        Wi2=np.asarray(inputs["Wi2"], np.float32).reshape(-1, 1),
        Wj1=np.asarray(inputs["Wj1"], np.float32),
        bj1=np.asarray(inputs["bj1"], np.float32).reshape(-1, 1),
        Wj2=np.asarray(inputs["Wj2"], np.float32).reshape(-1, 1),
    )
    maps = []
    for c in range(ncc):
        m = dict(common)
        m["attr_t"] = pre["attr_t"][c]
        m["src_t"] = pre["src_t"][c]
        m["slot_t"] = pre["slot_t"][c]
        m["h0_own"] = h0_own[c]
        maps.append(m)
    return maps


_prog_cache = {}


def run(cfg, inputs, run_fn=None):
    for name in ("b3", "bias"):
        assert np.allclose(np.asarray(inputs[name]), 0.0), f"{name} must be 0"
    pre = preprocess(cfg, inputs["x"], inputs["edge_index"], inputs["edge_attr"])
    bi2 = float(np.asarray(inputs["bi2"]).reshape(-1)[0])
    bj2 = float(np.asarray(inputs["bj2"]).reshape(-1)[0])
    key = (tuple(sorted(cfg.items())), pre["K"], bi2, bj2)
    if key not in _prog_cache:
        _prog_cache[key] = build_program(cfg, pre["K"], bi2, bj2)
    nc = _prog_cache[key]
    maps = make_in_maps(cfg, pre, inputs)
    if run_fn is not None:
        results = run_fn(nc, maps)
    else:
        results = run_bass_kernel_spmd(
            nc, maps, list(range(cfg["n_cores"]))
        ).results
    total = sum(float(r["partial"].reshape(-1)[0]) for r in results)
    return np.float32(total)


def kernel(**inputs):
    return run(FULL_CFG, inputs)

